# revision 4
# baseline (speedup 1.0000x reference)
"""Trainium2 Bass kernel for nn_ConceptIntergation (histogram_binning).

Reference computation:
    counts[b,s,n] = sum_k one_hot(concepts[b,s,k], 129)[..., n]  (n < 128; 128 = padding)
    out[b,s,n,d]  = counts[b,s,n] * emb_table[n,d]

Strategy (data-parallel over batch, 8 cores):
  - Each core handles B_LOC=8 batches -> 1600 (b,s) rows, output shard
    [1600, 128*64] f32 (~52 MB). The kernel is HBM-write bound; the whole
    design keeps the 16 SDMA store engines saturated from ~4us to the end.
  - Rows are processed in 128-row blocks (rows on partitions). Histogram via
    iota-compare on DVE (tensor_scalar is_equal + scalar_tensor_tensor
    accumulate), then broadcast tensor_tensor multiplies produce
    [128, 2048] chunks = counts[:,n] * emb[n,d]; each chunk is a 1 MB DMA
    store (contiguous 8 KB per partition).
  - HBM traffic is (almost) stores only: the 128-partition emb replica is
    NOT loaded from HBM (that would be 4 MB = ~10 us of the ~150 us HBM
    budget). Instead chunk 0 (1 MB) loads during the otherwise-idle ramp,
    and chunks 1..3 are replicated on-chip by the idle TensorEngine
    (ones[1,128]^T @ emb1[1,512] outer products -> PSUM -> ScalarE copy).
  - SDMA engines 7 and 15 are intermittently ~10-15% slower than the rest
    (known trn2 behavior). Engine load is set by the SBUF partition->port
    swizzle (port = bits[4:2]<<1 | bit[6]), so the 64 remainder rows
    (1600 = 12*128 + 64) are placed on partitions 0..63 (compute APs must
    start on a 32-partition quadrant), which avoids the partitions engines
    7/15 serve ({76-79,108-111} / {92-95,124-127}). Engines 7/15 then
    carry 12 rows/partition vs 13 on the even-port engines.
"""

import numpy as np

import concourse.bass as bass
import concourse.mybir as mybir
from concourse import bacc
from concourse.tile import TileContext
from concourse.bass_utils import run_bass_kernel_spmd

B, S, K = 64, 200, 4
N, D = 128, 64
ND = N * D                      # 8192
NCORES = 8
B_LOC = B // NCORES             # 8
ROWS = B_LOC * S                # 1600 (b,s) rows per core
P = 128
NFULL = ROWS // P               # 12 full blocks
REM = ROWS - NFULL * P          # 64 remainder rows
REM_P0 = 0                      # remainder lives on partitions 0..63
NBLK = NFULL + 1                # 13

CH = 4                          # emb/mul/store chunks per block
CW = ND // CH                   # 2048 cols per chunk (= 32 n-rows), 1 MB stores
NCH = N // CH                   # 32 n-rows per chunk
MMW = 512                       # matmul moving-dim width (HW max)

_NC_CACHE = {}


def _build_nc():
    nc = bacc.Bacc()
    idx = nc.declare_dram_parameter("idx", [P, NBLK * K], mybir.dt.float32, isOutput=False)
    embone = nc.declare_dram_parameter("embone", [1, ND], mybir.dt.float32, isOutput=False)
    embmini = nc.declare_dram_parameter("embmini", [P, CW], mybir.dt.float32, isOutput=False)
    iota = nc.declare_dram_parameter("iota", [P, N], mybir.dt.float32, isOutput=False)
    out = nc.declare_dram_parameter("out", [ROWS, ND], mybir.dt.float32, isOutput=True)

    with TileContext(nc) as tc:
        with (
            tc.tile_pool(name="const", bufs=1) as cpool,
            tc.tile_pool(name="counts", bufs=NBLK) as hpool,
            tc.tile_pool(name="work", bufs=12) as wpool,
            tc.psum_pool(name="psum", bufs=4) as ppool,
        ):
            # small inputs first so the first histogram can start immediately
            iota_sb = cpool.tile([P, N], mybir.dt.float32)
            nc.sync.dma_start(out=iota_sb, in_=iota[:, :])
            idx_sb = cpool.tile([P, NBLK * K], mybir.dt.float32)
            nc.sync.dma_start(out=idx_sb, in_=idx[:, :])
            # one-row emb copy (32 KB) for the on-chip TensorE broadcast
            emb1_sb = cpool.tile([1, ND], mybir.dt.float32)
            nc.sync.dma_start(out=emb1_sb, in_=embone[:, :])
            # chunk 0 of the 128-partition replica rides the idle ramp (1 MB)
            emb_sb = cpool.tile([P, ND], mybir.dt.float32)
            nc.sync.dma_start(out=emb_sb[:, 0:CW], in_=embmini[:, :])

            ones_sb = cpool.tile([1, P], mybir.dt.float32)
            nc.vector.memset(ones_sb, 1.0)

            # chunks 1..3 of the emb replica: outer-product broadcast on the
            # (otherwise idle) TensorEngine, drained PSUM->SBUF by ScalarE.
            for c in range(1, CH):
                for s in range(CW // MMW):
                    col = c * CW + s * MMW
                    pt = ppool.tile([P, MMW], mybir.dt.float32, tag="pt")
                    nc.tensor.matmul(
                        pt[:, :],
                        lhsT=ones_sb[:, :],
                        rhs=emb1_sb[:, col : col + MMW],
                        start=True,
                        stop=True,
                    )
                    nc.scalar.copy(out=emb_sb[:, col : col + MMW], in_=pt[:, :])

            def emit_hist(j, counts):
                p0, p1 = (0, P) if j < NFULL else (REM_P0, REM_P0 + REM)
                nc.vector.tensor_scalar(
                    out=counts[p0:p1],
                    in0=iota_sb[p0:p1],
                    scalar1=idx_sb[p0:p1, j * K : j * K + 1],
                    scalar2=None,
                    op0=mybir.AluOpType.is_equal,
                )
                for k in range(1, K):
                    nc.vector.scalar_tensor_tensor(
                        out=counts[p0:p1],
                        in0=iota_sb[p0:p1],
                        scalar=idx_sb[p0:p1, j * K + k : j * K + k + 1],
                        in1=counts[p0:p1],
                        op0=mybir.AluOpType.is_equal,
                        op1=mybir.AluOpType.add,
                    )

            def emit_mul(j, c, counts, split=1):
                p0, p1 = (0, P) if j < NFULL else (REM_P0, REM_P0 + REM)
                pj = p1 - p0
                r0 = j * P  # DRAM row of partition p0
                ot = wpool.tile([P, CW], mybir.dt.float32, tag="ot")
                w = CW // split
                nw = NCH // split
                for s in range(split):
                    nc.vector.tensor_tensor(
                        out=ot[p0:p1, s * w : (s + 1) * w].rearrange(
                            "p (n d) -> p n d", d=D
                        ),
                        in0=counts[
                            p0:p1, c * NCH + s * nw : c * NCH + (s + 1) * nw, None
                        ].broadcast_to([pj, nw, D]),
                        in1=emb_sb[p0:p1, c * CW + s * w : c * CW + (s + 1) * w].rearrange(
                            "p (n d) -> p n d", d=D
                        ),
                        op=mybir.AluOpType.mult,
                    )
                    nc.sync.dma_start(
                        out=out[r0 : r0 + pj, c * CW + s * w : c * CW + (s + 1) * w],
                        in_=ot[p0:p1, s * w : (s + 1) * w],
                    )

            # chunk-major: the c=0 stripe (gated only on the 1 MB mini load)
            # runs first; histograms are interleaved into it. Block 0 is
            # split into 512-col pieces so the first store issues ASAP.
            counts_tiles = [None] * NBLK
            for j in range(NBLK):
                counts = hpool.tile([P, N], mybir.dt.float32, tag="counts")
                counts_tiles[j] = counts
                emit_hist(j, counts)
                emit_mul(j, 0, counts, split=4 if j == 0 else 1)
            for c in range(1, CH):
                for j in range(NBLK):
                    emit_mul(j, c, counts_tiles[j])

    nc.finalize()
    return nc


def _get_nc():
    if "nc" not in _NC_CACHE:
        _NC_CACHE["nc"] = _build_nc()
    return _NC_CACHE["nc"]


def _prepare_in_maps(concepts, emb_table):
    concepts = np.asarray(concepts)
    emb = np.ascontiguousarray(np.asarray(emb_table, dtype=np.float32).reshape(1, ND))

    # per-core index shards laid out [P, NBLK*K]; full blocks j<NFULL put row
    # j*128+p on partition p, the 64 remainder rows sit on partitions 12..75.
    conc = concepts.reshape(NCORES, ROWS, K).astype(np.float32)
    idx_dev = np.full((NCORES, P, NBLK * K), float(N), dtype=np.float32)
    full = conc[:, : NFULL * P].reshape(NCORES, NFULL, P, K)
    idx_dev[:, :, : NFULL * K] = full.transpose(0, 2, 1, 3).reshape(NCORES, P, NFULL * K)
    idx_dev[:, REM_P0 : REM_P0 + REM, NFULL * K :] = conc[:, NFULL * P :]
    idx_dev = np.ascontiguousarray(idx_dev)

    iota = np.ascontiguousarray(
        np.broadcast_to(np.arange(N, dtype=np.float32), (P, N))
    )
    embmini = np.ascontiguousarray(np.broadcast_to(emb[:, :CW], (P, CW)))
    return [
        {"idx": idx_dev[i], "embone": emb, "embmini": embmini, "iota": iota}
        for i in range(NCORES)
    ]


def _run(concepts, emb_table, **spmd_kwargs):
    nc = _get_nc()
    in_maps = _prepare_in_maps(concepts, emb_table)
    res = run_bass_kernel_spmd(nc, in_maps, core_ids=list(range(NCORES)), **spmd_kwargs)
    out = np.concatenate(
        [res.results[i]["out"].reshape(B_LOC, S, N, D) for i in range(NCORES)],
        axis=0,
    )
    return out, res


def kernel(concepts, emb_table):
    out, _ = _run(concepts, emb_table)
    return out
